# revision 90
# baseline (speedup 1.0000x reference)
"""Trainium2 Bass kernel for nn_Block_40742059770386 (dense_cnn).

Per-sample adaptively-mixed, style-modulated, demodulated 3x3 conv
(StyleGAN2-style) + channel RMS norm + SiLU.

Sharding: data-parallel over batch. B=16 samples -> 8 cores x 2 samples.
The small kernel bank (2 x 256 x 256 x 3 x 3) and gamma are replicated.

v4 design (host does all per-sample prep; device = conv + norm):
  - the HOST premixes the softmax weight bank (a0*W0+a1*W1, same DMA
    bytes as the raw bank) AND modulates+pads x into [128, 66, 66]
    tiles (+6% x bytes): the device-side weight mix, pad-copy, and
    border memsets all vanish. First conv matmul fires at ~1.9us.
  - all staging bf16; conv = implicit GEMM over the DMA-filled padded
    tiles, 18 bf16 matmuls (full PE rate) per (512-px tile, output
    half). PE busy ~125.7us of ~135us total (93%).
  - DMA transfers serialize on the ISSUING engine (sync->SP,
    gpsimd->Pool), ~1.6us issue+sem latency per hop: weights ride SP,
    x/smat ride Pool; first x chunk splits across both streams.
  - the PE p-state ramp (0.65/1.2GHz until ~3us wall) is paid by two
    junk matmuls ahead of the first conv chain.
  - demod d[o] via host Gram stats (smat) + tiny matvecs (deferred
    behind the first conv chain); d folds into ACT Square's scale,
    d*gamma into ACT Copy's.
  - channel norm sums: steady groups use Pool partition_all_reduce +
    Pool add + row-gather DMA into SBUF [G, 512] (PE-free); the two
    tail groups use one-hot-column matmuls into PSUM.
  - one batched rsqrt chain per group (DVE cost is free-size only):
    fp32 bit-trick+Newton steady, bf16 for the tail groups.
  - inv rows broadcast p0->all via DRAM bounce in steady state
    (z = yc*invb on Pool, all-SBUF); via a PE ones-matmul for the
    final tile. BIR rules: GPSIMD never touches PSUM; an op may read
    at most ONE input from PSUM.
  - sample s+1's prologue is emitted before sample s's deferred last
    finish (no boundary idle); last sample's groups are [5, 2, 1].
"""

import numpy as np

import concourse.bass as bass
import concourse.bacc as bacc
import concourse.mybir as mybir
import concourse.tile as tile
from contextlib import ExitStack
from concourse.bass_utils import run_bass_kernel_spmd
from concourse import bass_isa

# ---- problem constants (hardcoded; kernel.py must be self-contained) ----
B, C_IN, C_OUT, H, W, K, NK = 16, 256, 256, 64, 64, 3, 2
EPS = 1e-8
N_CORES = 8
S = B // N_CORES            # samples per core
PB = 128                    # partitions per block
IB = C_IN // PB             # input channel blocks
OB = C_OUT // PB            # output channel blocks
HW = H * W                  # 4096
PADH, PADW = H + 2, W + 2   # 66, 66
PT = 512                    # pixels per tile (one PSUM bank of fp32)
ROWS_PT = PT // W           # 8 rows per pixel tile
NPT = HW // PT              # 8 pixel tiles
KK = K * K                  # 9
NVEC = 2 + IB + 3 * IB      # packed per-sample vector columns
RT_CLAMP = 1e-24            # clamp on the norm-square row

F32 = mybir.dt.float32
BF16 = mybir.dt.bfloat16
I32 = mybir.dt.int32
I16 = mybir.dt.int16

AF = mybir.ActivationFunctionType
ALU = mybir.AluOpType
MAGIC = 0x5F3759DF
MAGIC16 = 0x5F37
# CoreSim does not implement Silu; decompose for sim-only runs
import os
SIM_SILU = os.environ.get("KERNEL_SIM_SILU", "0") == "1"


def _newton_rsqrt_steps(nc, pool, r, x, shape, tag, iters):
    """Refine r ~ rsqrt(x): r' = r * (1.5 - 0.5 * x * r^2). Returns tile."""
    xh = pool.tile(shape, F32, tag=f"{tag}_xh", name=f"{tag}_xh")
    nc.vector.tensor_scalar_mul(out=xh, in0=x, scalar1=0.5)
    for it in range(iters):
        t = pool.tile(shape, F32, tag=f"{tag}_t{it}", name=f"{tag}_t{it}")
        nc.vector.tensor_mul(out=t, in0=r, in1=r)
        nc.vector.tensor_mul(out=t, in0=t, in1=xh)
        nc.vector.tensor_scalar(
            out=t, in0=t, scalar1=-1.0, scalar2=1.5, op0=ALU.mult, op1=ALU.add
        )
        r2 = pool.tile(shape, F32, tag=f"{tag}_r{it}", name=f"{tag}_r{it}")
        nc.vector.tensor_mul(out=r2, in0=r, in1=t)
        r = r2
    return r


def _rsqrt_dve(nc, pool, src_ap, clamp, shape, tag, iters=2):
    """rsqrt(max(src, clamp)) entirely on DVE: bit-trick seed + Newton."""
    x = pool.tile(shape, F32, tag=f"{tag}_x", name=f"{tag}_x")
    nc.vector.tensor_scalar_max(out=x, in0=src_ap, scalar1=float(clamp))
    seed = pool.tile(shape, I32, tag=f"{tag}_s", name=f"{tag}_s")
    nc.vector.tensor_scalar(
        out=seed, in0=x.bitcast(I32), scalar1=1, scalar2=None,
        op0=ALU.logical_shift_right,
    )                                   # bits >> 1
    nc.vector.tensor_scalar(
        out=seed, in0=seed, scalar1=-1, scalar2=MAGIC,
        op0=ALU.mult, op1=ALU.add,
    )                                   # MAGIC - (bits >> 1)
    return _newton_rsqrt_steps(nc, pool, seed.bitcast(F32), x, shape, tag, iters=iters)


def _rsqrt_bf16(nc, pool, src_ap, clamp, shape, tag):
    """Fast rsqrt(max(src, clamp)) -> bf16; bit-trick seed + 1 bf16 Newton.

    ~0.3% error from bf16 roundings in the Newton bracket -- use only for
    the small final pixel group (shortest possible tail chain).
    """
    x = pool.tile(shape, BF16, tag=f"{tag}_x", name=f"{tag}_x")
    nc.vector.tensor_scalar_max(out=x, in0=src_ap, scalar1=float(clamp))
    seed = pool.tile(shape, I16, tag=f"{tag}_s", name=f"{tag}_s")
    nc.vector.tensor_scalar(
        out=seed, in0=x.bitcast(I16), scalar1=1, scalar2=None,
        op0=ALU.logical_shift_right,
    )
    nc.vector.tensor_scalar(
        out=seed, in0=seed, scalar1=-1, scalar2=MAGIC16,
        op0=ALU.mult, op1=ALU.add,
    )
    r = seed.bitcast(BF16)
    xh = pool.tile(shape, BF16, tag=f"{tag}_xh", name=f"{tag}_xh")
    nc.vector.tensor_scalar_mul(out=xh, in0=x, scalar1=0.5)
    t = pool.tile(shape, BF16, tag=f"{tag}_t", name=f"{tag}_t")
    nc.vector.tensor_mul(out=t, in0=r, in1=r)
    nc.vector.tensor_mul(out=t, in0=t, in1=xh)
    nc.vector.tensor_scalar(
        out=t, in0=t, scalar1=-1.0, scalar2=1.5, op0=ALU.mult, op1=ALU.add
    )
    r2 = pool.tile(shape, BF16, tag=f"{tag}_r2", name=f"{tag}_r2")
    nc.vector.tensor_mul(out=r2, in0=r, in1=t)
    return r2


def build_program():
    nc = bacc.Bacc(trn_type="TRN2", debug=False)

    x_d = nc.declare_dram_parameter("x", [S, IB, PB, PADH * PADW], BF16, isOutput=False)
    wt_d = nc.declare_dram_parameter("wT", [S, IB, PB, C_OUT, KK], BF16, isOutput=False)
    vecs_d = nc.declare_dram_parameter("vecs", [S, PB, NVEC], F32, isOutput=False)
    smat_d = nc.declare_dram_parameter("smat", [PB, 3, IB, C_OUT], F32, isOutput=False)
    g16_d = nc.declare_dram_parameter("g16", [OB, PB, 1], F32, isOutput=False)
    hotr_d = nc.declare_dram_parameter("hotr", [4, 3, PB], BF16, isOutput=False)
    y_d = nc.declare_dram_parameter("y", [S, OB, PB, HW], BF16, isOutput=True)

    with ExitStack() as ctx:
        tc = ctx.enter_context(tile.TileContext(nc))
        const = ctx.enter_context(tc.tile_pool(name="const", bufs=1))
        wpool = ctx.enter_context(tc.tile_pool(name="wmix", bufs=3))
        xrp = ctx.enter_context(tc.tile_pool(name="xpad", bufs=3))
        small = ctx.enter_context(tc.tile_pool(name="small", bufs=4))
        sq_p = ctx.enter_context(tc.tile_pool(name="sqp", bufs=4))
        ycp = ctx.enter_context(tc.tile_pool(name="ycpool", bufs=12))
        invp = ctx.enter_context(tc.tile_pool(name="invp", bufs=2))
        nsgp = ctx.enter_context(tc.tile_pool(name="nsgp", bufs=2))
        nstp = ctx.enter_context(tc.tile_pool(name="nstp", bufs=3))
        bcastp = ctx.enter_context(tc.tile_pool(name="bcast", bufs=4))
        outp = ctx.enter_context(tc.tile_pool(name="outs", bufs=3))
        dramp = ctx.enter_context(tc.tile_pool(name="dram", bufs=2, space="DRAM"))
        pconv = ctx.enter_context(tc.tile_pool(name="pconv", bufs=4, space="PSUM"))
        pnorm = ctx.enter_context(tc.tile_pool(name="pnorm", bufs=2, space="PSUM"))
        pbc = ctx.enter_context(tc.tile_pool(name="pbc", bufs=1, space="PSUM"))
        pdsq = ctx.enter_context(tc.tile_pool(name="pdsq", bufs=1, space="PSUM"))

        # ---- resident constants ----
        g16sb = [const.tile([PB, 1], F32, tag=f"g16_{ob}", name=f"g16_{ob}")
                 for ob in range(OB)]
        # one-hot column tiles for norm-row accumulation (lhsT of the
        # channel-sum matmul; row j of the PSUM [G, 512] gets the sum).
        # Pixel-tile groups per sample: s0 balanced, s1 tail-light with
        # shrinking groups (short exposed chain after the last matmul).
        GROUPS = {s: [4, 4] for s in range(S)}
        GROUPS[S - 1] = [5, 2, 1]
        # tail groups: fast bf16 Newton chains; B broadcasts via a bf16
        # DMA bounce (z stays on Pool), C via a PE ones-matmul (shortest
        # serial chain after the final conv matmul)
        FASTG = {(S - 1, 1): "fastdma", (S - 1, 2): "fastpe"}
        hots = {}
        for (s_, g_) in FASTG:
            G = GROUPS[s_][g_]
            for j in range(G):
                if (G, j) in hots:
                    continue
                hj = const.tile([PB, G], BF16, tag=f"hot{G}_{j}", name=f"hot{G}_{j}")
                nc.vector.memset(hj, 0.0)
                nc.vector.memset(hj[:, j:j + 1], 1.0)
                hots[(G, j)] = hj
        smat_t = const.tile([PB, 3, IB, C_OUT], F32, tag="smat", name="smat")
        # PE clock warm-up fodder: the p-state ramp (0.65/1.2GHz for the
        # first ~3us of activity) is paid by junk matmuls during the
        # DMA-bound head instead of by the first real conv chain
        junk = const.tile([PB, PT], BF16, tag="junk", name="junk")
        nc.vector.memset(junk, 0.0)
        N_WARM = int(os.environ.get("KERNEL_WARM", "2"))
        # row-hot tiles: [G, 128] with row j all-ones -- lhsT of the
        # PE broadcast matmul (out[o,:] = inv[j,:]) for the tail groups.
        # Row-wise memsets can't start mid-partition; DMA'd from host.
        hotr = {}
        hidx = 0
        for (s_, g_), mode in sorted(FASTG.items()):
            if mode != "fastpe":
                continue
            G = GROUPS[s_][g_]
            for j in range(G):
                if (G, j) in hotr:
                    continue
                hr = const.tile([G, PB], BF16, tag=f"hotr{G}_{j}",
                                name=f"hotr{G}_{j}")
                nc.gpsimd.dma_start(out=hr, in_=hotr_d[hidx, 0:G])
                hotr[(G, j)] = hr
                hidx += 1

        HH = H // 2
        QC = HW // 4          # x quarter, in flat columns

        def prologue(s):
            """Per-sample setup: DMAs, weight mix, x pad-scale, demod.

            Emission order IS engine order (in-order engines): weights on
            the SP DMA stream, x + smat on the Pool stream; DVE does mix
            o-half-0, x quarter 0, demod rsqrt, then the rest.
            """
            st = {}
            vec = small.tile([PB, NVEC], F32, tag="vec", name="vec")
            nc.sync.dma_start(out=vec, in_=vecs_d[s])
            st["mpc"] = [vec[:, 2 + ib:3 + ib] for ib in range(IB)]
            m2k = [[vec[:, 4 + 3 * ib + k:5 + 3 * ib + k] for k in range(3)]
                   for ib in range(IB)]

            # host-premixed per-sample weights, straight into SBUF via
            # the SP stream: o-half 0 first (first conv chains), ib-major
            wmix = [wpool.tile([PB, C_OUT, KK], BF16, tag="wmix", name="wmix")
                    for _ in range(IB)]
            st["wmix"] = wmix
            for ib in range(IB):
                nc.sync.dma_start(
                    out=wmix[ib][:, 0:PB, :], in_=wt_d[s, ib, :, 0:PB, :],
                )
            xp = []
            for ib in range(IB):
                xp.append(xrp.tile([PB, PADH, PADW], BF16, tag="xpad",
                                   name="xpad"))
            st["xp"] = xp

            # x quarters: Pool stream for s0 (SP is busy with weights),
            # SP for later samples (Pool carries y writebacks by then).
            # s0's first quarter is split across BOTH streams so the two
            # ib blocks land (and the pad-copy can start) ~0.8us sooner.
            xdma = nc.gpsimd.dma_start if s == 0 else nc.sync.dma_start
            xfs = [xfp.tile([PB, HW], BF16, tag="xf", name="xf")
                   for _ in range(IB)]
            nc.gpsimd.dma_start(out=xfs[0][:, 0:QC], in_=x_d[s, 0, :, 0:QC])
            for ib in range(IB):
                xr = xp[ib]
                nc.gpsimd.memset(xr[:, 0:1, :], 0.0)
                nc.gpsimd.memset(xr[:, PADH - 1:PADH, :], 0.0)
                nc.gpsimd.memset(xr[:, 1:H + 1, 0:1], 0.0)
                nc.gpsimd.memset(xr[:, 1:H + 1, PADW - 1:PADW], 0.0)
            (nc.sync.dma_start if s == 0 else nc.gpsimd.dma_start)(
                out=xfs[1][:, 0:QC], in_=x_d[s, 1, :, 0:QC])
            if s == 0:
                # demod stats + gamma, one transfer each, on Pool
                nc.gpsimd.dma_start(out=smat_t, in_=smat_d[0:PB])
                nc.gpsimd.dma_start(out=g16sb[0], in_=g16_d[0])
                nc.gpsimd.dma_start(out=g16sb[1], in_=g16_d[1])
            for ib in range(IB):
                nc.sync.dma_start(
                    out=wmix[ib][:, PB:C_OUT, :], in_=wt_d[s, ib, :, PB:C_OUT, :],
                )
            for q in range(1, 4):
                for ib in range(IB):
                    xdma(out=xfs[ib][:, q * QC:(q + 1) * QC],
                         in_=x_d[s, ib, :, q * QC:(q + 1) * QC])

            st["m2k"] = m2k

            if s > 0:
                emit_demod(st)

            for q in range(1, 4):
                r0, r1 = XQ[q]
                for ib in range(IB):
                    xdma(out=xp[ib][:, r0:r1, :],
                         in_=x_d[s, ib, :, r0 * PADW:r1 * PADW])
            return st

        def emit_demod(st):
            """Demod matvecs (PE) + d/gd columns (DVE).

            Deferred past the first conv chain for sample 0 so the
            PE's in-order queue isn't blocked waiting on smat."""
            m2k = st["m2k"]
            dsq_ps = [pdsq.tile([PB, 1], F32, tag="dsq", name="dsq")
                      for _ in range(OB)]
            for ob in range(OB):
                i_mv = 0
                for ib in range(IB):
                    for k in range(3):
                        nc.tensor.matmul(
                            dsq_ps[ob],
                            lhsT=smat_t[:, k, ib, ob * PB:(ob + 1) * PB],
                            rhs=m2k[ib][k],
                            start=(i_mv == 0), stop=(i_mv == 3 * IB - 1),
                        )
                        i_mv += 1
            dcol, gdcol = [], []
            for ob in range(OB):
                d = _rsqrt_dve(nc, small, dsq_ps[ob], EPS, [PB, 1], f"d{ob}",
                               iters=2)
                dcol.append(d)
                gd = small.tile([PB, 1], F32, tag=f"gd{ob}", name=f"gd{ob}")
                nc.vector.tensor_mul(out=gd, in0=d, in1=g16sb[ob])
                gdcol.append(gd)
            st["dcol"], st["gdcol"] = dcol, gdcol

        def conv_group(s, st, g):
            """Emit conv + square + channel-norm sums for pixel group g.

            Tail groups accumulate the 256-channel sums on PE (one-hot
            matmuls into PSUM [G, 512]); steady groups use Pool's
            partition_all_reduce + add + a row-gather DMA into an SBUF
            [G, 512], keeping the PE free for conv.
            """
            G = GROUPS[s][g]
            g0 = sum(GROUPS[s][:g])
            mode = FASTG.get((s, g), "slow")
            if mode == "slow":
                nsum = nsgp.tile([G, PT], BF16, tag="nsg", name="nsg")
            else:
                nsum = pnorm.tile([G, PT], F32, tag="nsum", name="nsum")
            ycs = {}
            for lpt in range(G):
                pt = g0 + lpt
                pss = []
                for ob in range(OB):
                    if s == 0 and g == 0 and lpt == 0 and ob == 0 and N_WARM:
                        wps = pconv.tile([PB, PT], F32, tag="conv", name="conv")
                        for i in range(N_WARM):
                            nc.tensor.matmul(
                                wps[0:2, :], lhsT=hots[(2, 0)], rhs=junk,
                                start=(i == 0), stop=(i == N_WARM - 1),
                            )
                    ps = pconv.tile([PB, PT], F32, tag="conv", name="conv")
                    pss.append(ps)
                    n_mm = IB * KK
                    i_mm = 0
                    for ib in range(IB):
                        for ki in range(K):
                            for kj in range(K):
                                lhsT = st["wmix"][ib][
                                    :, ob * PB:(ob + 1) * PB, ki * K + kj]
                                rhs = st["xp"][ib][
                                    :,
                                    pt * ROWS_PT + ki: pt * ROWS_PT + ki + ROWS_PT,
                                    kj: kj + W,
                                ]
                                nc.tensor.matmul(
                                    ps, lhsT=lhsT, rhs=rhs,
                                    start=(i_mm == 0), stop=(i_mm == n_mm - 1),
                                )
                                i_mm += 1
                    if s == 0 and g == 0 and lpt == 0 and ob == 0:
                        # demod slots in behind the first conv chain
                        emit_demod(st)
                # ACT order: both squares first (they gate the norm-sum
                # matmuls and, at the tail, the final rsqrt chain)
                sqs = []
                for ob in range(OB):
                    sq = sq_p.tile([PB, PT], BF16, tag="sq", name="sq")
                    nc.scalar.activation(
                        out=sq, in_=pss[ob], func=AF.Square, scale=st["dcol"][ob]
                    )
                    sqs.append(sq)
                if mode == "slow":
                    for ob in range(OB):
                        nc.gpsimd.partition_all_reduce(
                            sqs[ob][:], sqs[ob][:], PB, bass_isa.ReduceOp.add
                        )
                    nst = nstp.tile([1, PT], BF16, tag="nst", name="nst")
                    nc.gpsimd.tensor_add(
                        out=nst, in0=sqs[0][0:1, :], in1=sqs[1][0:1, :]
                    )
                    nc.sync.dma_start(out=nsum[lpt:lpt + 1, :], in_=nst)
                else:
                    for ob in range(OB):
                        nc.tensor.matmul(
                            nsum, lhsT=hots[(G, lpt)], rhs=sqs[ob],
                            start=(lpt == 0 and ob == 0),
                            stop=(lpt == G - 1 and ob == OB - 1),
                        )
                for ob in range(OB):
                    # y*d*gamma*sqrt(C) kept fp32 for the output path
                    yc = ycp.tile([PB, PT], F32, tag="yc", name="yc")
                    nc.scalar.activation(
                        out=yc, in_=pss[ob], func=AF.Copy, scale=st["gdcol"][ob]
                    )
                    ycs[(lpt, ob)] = yc
            return nsum, ycs

        def finish_group(s, g, nsum, ycs, st=None):
            """rsqrt of the norm rows + z/SiLU/writeback for group g.

            Steady state broadcasts inv rows p0->all via a DRAM bounce
            (latency hidden under conv); the tail groups (FASTG) use a
            PE ones-matmul broadcast instead (DMA sem latency is ~1.6us
            per hop, matmul is ~0.3us total).
            """
            G = GROUPS[s][g]
            g0 = sum(GROUPS[s][:g])
            mode = FASTG.get((s, g), "slow")
            if mode == "fastpe":
                p = g0
                inv = _rsqrt_bf16(nc, invp, nsum, RT_CLAMP, [G, PT], "nrmf")
                invb = pbc.tile([PB, PT], F32, tag="invb_ps", name="invb_ps")
                nc.tensor.matmul(
                    invb, lhsT=hotr[(G, 0)], rhs=inv, start=True, stop=True,
                )
                for ob in range(OB):
                    # BIR: an op may read only ONE input from PSUM --
                    # yc is the SBUF copy, invb the PE-broadcast PSUM
                    z = outp.tile([PB, PT], F32, tag="z", name="z")
                    nc.vector.tensor_mul(out=z, in0=ycs[(0, ob)], in1=invb)
                    yo = outp.tile([PB, PT], BF16, tag="yo", name="yo")
                    if SIM_SILU:
                        nc.scalar.activation(out=yo, in_=z, func=AF.Sigmoid)
                        nc.vector.tensor_mul(out=yo, in0=z, in1=yo)
                    else:
                        nc.scalar.activation(out=yo, in_=z, func=AF.Silu)
                    nc.sync.dma_start(
                        out=y_d[s, ob, :, p * PT:(p + 1) * PT], in_=yo,
                    )
                return
            if mode == "slow":
                inv = _rsqrt_dve(nc, invp, nsum, RT_CLAMP, [G, PT], "nrm",
                                 iters=1)
            else:
                inv = _rsqrt_bf16(nc, invp, nsum, RT_CLAMP, [G, PT], "nrmf")
            idt = BF16 if mode == "fastdma" else F32
            dinv = dramp.tile([G, PT], idt, tag="dinv", name="dinv")
            nc.sync.dma_start(out=dinv, in_=inv)
            last = (s == S - 1 and g == len(GROUPS[s]) - 1)
            for lpt in range(G):
                p = g0 + lpt
                invb = bcastp.tile([PB, PT], idt, tag="invb", name="invb")
                nc.sync.dma_start(
                    out=invb, in_=dinv[lpt:lpt + 1, :].to_broadcast((PB, PT)),
                )
                for ob in range(OB):
                    z = outp.tile([PB, PT], F32, tag="z", name="z")
                    if False:
                        pass
                    else:
                        # all-SBUF operands -> Pool (427ns, cheaper than
                        # DVE and off the critical engines)
                        nc.gpsimd.tensor_mul(out=z, in0=ycs[(lpt, ob)], in1=invb)
                    yo = outp.tile([PB, PT], BF16, tag="yo", name="yo")
                    if SIM_SILU:
                        nc.scalar.activation(out=yo, in_=z, func=AF.Sigmoid)
                        nc.vector.tensor_mul(out=yo, in0=z, in1=yo)
                    else:
                        nc.scalar.activation(out=yo, in_=z, func=AF.Silu)
                    ydma = nc.sync.dma_start if last else nc.gpsimd.dma_start
                    ydma(out=y_d[s, ob, :, p * PT:(p + 1) * PT], in_=yo)

        # ---- main schedule: the prologue of sample s+1 is emitted before
        # the deferred finish of sample s's final group, so the in-order
        # DVE/PE streams roll into the next sample without idling. On the
        # last sample the final two (small) groups' finishes are emitted
        # after ALL conv work, so PE never waits on a norm chain.
        st = prologue(0)
        for s in range(S):
            ngroups = len(GROUPS[s])
            if s < S - 1:
                for g in range(ngroups):
                    nsum, ycs = conv_group(s, st, g)
                    if g == ngroups - 1:
                        nst = prologue(s + 1)
                        finish_group(s, g, nsum, ycs)
                        st = nst
                    else:
                        finish_group(s, g, nsum, ycs)
            else:
                pend = []
                for g in range(ngroups):
                    nsum, ycs = conv_group(s, st, g)
                    if g < ngroups - 2:
                        finish_group(s, g, nsum, ycs)
                    else:
                        pend.append((g, nsum, ycs))
                for g, nsum, ycs in pend:
                    finish_group(s, g, nsum, ycs, st=st)
    nc.finalize()
    return nc


_NC_CACHE = {}


def _get_program():
    if "nc" not in _NC_CACHE:
        _NC_CACHE["nc"] = build_program()
    return _NC_CACHE["nc"]


def _host_prep(x, mod, kernel_mod, weights, gamma):
    import ml_dtypes

    x = np.asarray(x, dtype=np.float32)
    mod = np.asarray(mod, dtype=np.float32)
    kernel_mod = np.asarray(kernel_mod, dtype=np.float32)
    weights = np.asarray(weights, dtype=np.float32)
    gamma = np.asarray(gamma, dtype=np.float32)

    # softmax over the (tiny) kernel bank dim
    e = np.exp(kernel_mod - kernel_mod.max(axis=-1, keepdims=True))
    attn = (e / e.sum(axis=-1, keepdims=True)).astype(np.float32)     # [B, NK]

    modp1 = mod + 1.0                                                 # [B, C_IN]
    m2 = modp1 * modp1

    # [NK, O, I, K, K] -> [NK, I, O, K*K]; per-sample softmax mix done on
    # the host (fp32) so the device never touches the raw bank
    wTf = weights.transpose(0, 2, 1, 3, 4).reshape(NK, IB, PB, C_OUT, KK)
    # bank Gram stats over kk: S00, S01, S11 as [i, o], split by i-block
    wio = weights.transpose(0, 2, 1, 3, 4).reshape(NK, C_IN, C_OUT, KK)
    s00 = (wio[0] * wio[0]).sum(-1)
    s01 = (wio[0] * wio[1]).sum(-1)
    s11 = (wio[1] * wio[1]).sum(-1)
    smat = np.ascontiguousarray(
        np.stack([s00, s01, s11]).reshape(3, IB, PB, C_OUT)
        .transpose(2, 0, 1, 3).astype(np.float32)
    )
    g16 = np.ascontiguousarray(
        (gamma * np.sqrt(C_OUT)).astype(np.float32).reshape(OB, PB, 1)
    )
    # row-hot lhsT patterns for the tail PE-broadcast: slot i = one-hot
    # row i (slot 0 also serves the G=1 all-ones case); consumed in
    # build_program's sorted-FASTG "fastpe" order
    hotr = np.zeros((4, 3, PB), ml_dtypes.bfloat16)
    hotr[0, 0, :] = 1
    hotr[1, 1, :] = 1
    hotr[2, 2, :] = 1
    hotr[3, 0, :] = 1

    in_maps = []
    for c in range(N_CORES):
        sl = slice(c * S, (c + 1) * S)
        vecs = np.empty((S, PB, NVEC), np.float32)
        for si in range(S):
            b = c * S + si
            a0, a1 = attn[b, 0], attn[b, 1]
            vecs[si, :, 0] = a0
            vecs[si, :, 1] = a1
            vecs[si, :, 2:2 + IB] = modp1[b].reshape(IB, PB).T
            m2b = m2[b].reshape(IB, PB)
            for ib in range(IB):
                vecs[si, :, 4 + 3 * ib] = m2b[ib] * (a0 * a0)
                vecs[si, :, 5 + 3 * ib] = m2b[ib] * (2.0 * a0 * a1)
                vecs[si, :, 6 + 3 * ib] = m2b[ib] * (a1 * a1)
        wmix = np.ascontiguousarray(
            attn[sl, 0, None, None, None, None] * wTf[0][None]
            + attn[sl, 1, None, None, None, None] * wTf[1][None]
        ).astype(ml_dtypes.bfloat16)                    # [S, IB, PB, C_OUT, KK]
        xpad = np.zeros((S, IB, PB, PADH, PADW), np.float32)
        xpad[:, :, :, 1:H + 1, 1:W + 1] = (
            x[sl] * modp1[sl, :, None, None]
        ).reshape(S, IB, PB, H, W)
        in_maps.append({
            "x": xpad.reshape(S, IB, PB, PADH * PADW).astype(ml_dtypes.bfloat16),
            "wT": wmix,
            "smat": smat,
            "hotr": hotr,
            "vecs": vecs,
            "g16": g16,
        })
    return in_maps


def kernel(x, mod, kernel_mod, weights, gamma, _trace=False, _trace_kwargs=None):
    nc = _get_program()
    in_maps = _host_prep(x, mod, kernel_mod, weights, gamma)
    res = run_bass_kernel_spmd(
        nc, in_maps, list(range(N_CORES)),
        trace=_trace, **(_trace_kwargs or {}),
    )
    y = np.concatenate(
        [np.asarray(res.results[c]["y"]).astype(np.float32).reshape(S, C_OUT, H, W)
         for c in range(N_CORES)],
        axis=0,
    )
    if _trace:
        kernel.last_results = res
    return y


kernel.last_results = None


# revision 91
# speedup vs baseline: 1.0035x; 1.0035x over previous
"""Trainium2 Bass kernel for nn_Block_40742059770386 (dense_cnn).

Per-sample adaptively-mixed, style-modulated, demodulated 3x3 conv
(StyleGAN2-style) + channel RMS norm + SiLU.

Sharding: data-parallel over batch. B=16 samples -> 8 cores x 2 samples.
The small kernel bank (2 x 256 x 256 x 3 x 3) and gamma are replicated.

v4 design (host does all per-sample prep; device = conv + norm):
  - the HOST premixes the softmax weight bank (a0*W0+a1*W1, same DMA
    bytes as the raw bank) AND modulates+pads x into [128, 66, 66]
    tiles (+6% x bytes): the device-side weight mix, pad-copy, and
    border memsets all vanish. First conv matmul fires at ~1.9us.
  - all staging bf16; conv = implicit GEMM over the DMA-filled padded
    tiles, 18 bf16 matmuls (full PE rate) per (512-px tile, output
    half). PE busy ~125.7us of ~135us total (93%).
  - DMA transfers serialize on the ISSUING engine (sync->SP,
    gpsimd->Pool), ~1.6us issue+sem latency per hop: weights ride SP,
    x/smat ride Pool; first x chunk splits across both streams.
  - the PE p-state ramp (0.65/1.2GHz until ~3us wall) is paid by two
    junk matmuls ahead of the first conv chain.
  - demod d[o] via host Gram stats (smat) + tiny matvecs (deferred
    behind the first conv chain); d folds into ACT Square's scale,
    d*gamma into ACT Copy's.
  - channel norm sums: steady groups use Pool partition_all_reduce +
    Pool add + row-gather DMA into SBUF [G, 512] (PE-free); the two
    tail groups use one-hot-column matmuls into PSUM.
  - one batched rsqrt chain per group (DVE cost is free-size only):
    fp32 bit-trick+Newton steady, bf16 for the tail groups.
  - inv rows broadcast p0->all via DRAM bounce in steady state
    (z = yc*invb on Pool, all-SBUF); via a PE ones-matmul for the
    final tile. BIR rules: GPSIMD never touches PSUM; an op may read
    at most ONE input from PSUM.
  - sample s+1's prologue is emitted before sample s's deferred last
    finish (no boundary idle); last sample's groups are [5, 2, 1].
"""

import numpy as np

import concourse.bass as bass
import concourse.bacc as bacc
import concourse.mybir as mybir
import concourse.tile as tile
from contextlib import ExitStack
from concourse.bass_utils import run_bass_kernel_spmd
from concourse import bass_isa

# ---- problem constants (hardcoded; kernel.py must be self-contained) ----
B, C_IN, C_OUT, H, W, K, NK = 16, 256, 256, 64, 64, 3, 2
EPS = 1e-8
N_CORES = 8
S = B // N_CORES            # samples per core
PB = 128                    # partitions per block
IB = C_IN // PB             # input channel blocks
OB = C_OUT // PB            # output channel blocks
HW = H * W                  # 4096
PADH, PADW = H + 2, W + 2   # 66, 66
PT = 512                    # pixels per tile (one PSUM bank of fp32)
ROWS_PT = PT // W           # 8 rows per pixel tile
NPT = HW // PT              # 8 pixel tiles
KK = K * K                  # 9
NVEC = 2 + IB + 3 * IB      # packed per-sample vector columns
RT_CLAMP = 1e-24            # clamp on the norm-square row

F32 = mybir.dt.float32
BF16 = mybir.dt.bfloat16
I32 = mybir.dt.int32
I16 = mybir.dt.int16

AF = mybir.ActivationFunctionType
ALU = mybir.AluOpType
MAGIC = 0x5F3759DF
MAGIC16 = 0x5F37
# CoreSim does not implement Silu; decompose for sim-only runs
import os
SIM_SILU = os.environ.get("KERNEL_SIM_SILU", "0") == "1"


def _newton_rsqrt_steps(nc, pool, r, x, shape, tag, iters):
    """Refine r ~ rsqrt(x): r' = r * (1.5 - 0.5 * x * r^2). Returns tile."""
    xh = pool.tile(shape, F32, tag=f"{tag}_xh", name=f"{tag}_xh")
    nc.vector.tensor_scalar_mul(out=xh, in0=x, scalar1=0.5)
    for it in range(iters):
        t = pool.tile(shape, F32, tag=f"{tag}_t{it}", name=f"{tag}_t{it}")
        nc.vector.tensor_mul(out=t, in0=r, in1=r)
        nc.vector.tensor_mul(out=t, in0=t, in1=xh)
        nc.vector.tensor_scalar(
            out=t, in0=t, scalar1=-1.0, scalar2=1.5, op0=ALU.mult, op1=ALU.add
        )
        r2 = pool.tile(shape, F32, tag=f"{tag}_r{it}", name=f"{tag}_r{it}")
        nc.vector.tensor_mul(out=r2, in0=r, in1=t)
        r = r2
    return r


def _rsqrt_dve(nc, pool, src_ap, clamp, shape, tag, iters=2):
    """rsqrt(max(src, clamp)) entirely on DVE: bit-trick seed + Newton."""
    x = pool.tile(shape, F32, tag=f"{tag}_x", name=f"{tag}_x")
    nc.vector.tensor_scalar_max(out=x, in0=src_ap, scalar1=float(clamp))
    seed = pool.tile(shape, I32, tag=f"{tag}_s", name=f"{tag}_s")
    nc.vector.tensor_scalar(
        out=seed, in0=x.bitcast(I32), scalar1=1, scalar2=None,
        op0=ALU.logical_shift_right,
    )                                   # bits >> 1
    nc.vector.tensor_scalar(
        out=seed, in0=seed, scalar1=-1, scalar2=MAGIC,
        op0=ALU.mult, op1=ALU.add,
    )                                   # MAGIC - (bits >> 1)
    return _newton_rsqrt_steps(nc, pool, seed.bitcast(F32), x, shape, tag, iters=iters)


def _rsqrt_bf16(nc, pool, src_ap, clamp, shape, tag):
    """Fast rsqrt(max(src, clamp)) -> bf16; bit-trick seed + 1 bf16 Newton.

    ~0.3% error from bf16 roundings in the Newton bracket -- use only for
    the small final pixel group (shortest possible tail chain).
    """
    x = pool.tile(shape, BF16, tag=f"{tag}_x", name=f"{tag}_x")
    nc.vector.tensor_scalar_max(out=x, in0=src_ap, scalar1=float(clamp))
    seed = pool.tile(shape, I16, tag=f"{tag}_s", name=f"{tag}_s")
    nc.vector.tensor_scalar(
        out=seed, in0=x.bitcast(I16), scalar1=1, scalar2=None,
        op0=ALU.logical_shift_right,
    )
    nc.vector.tensor_scalar(
        out=seed, in0=seed, scalar1=-1, scalar2=MAGIC16,
        op0=ALU.mult, op1=ALU.add,
    )
    r = seed.bitcast(BF16)
    xh = pool.tile(shape, BF16, tag=f"{tag}_xh", name=f"{tag}_xh")
    nc.vector.tensor_scalar_mul(out=xh, in0=x, scalar1=0.5)
    t = pool.tile(shape, BF16, tag=f"{tag}_t", name=f"{tag}_t")
    nc.vector.tensor_mul(out=t, in0=r, in1=r)
    nc.vector.tensor_mul(out=t, in0=t, in1=xh)
    nc.vector.tensor_scalar(
        out=t, in0=t, scalar1=-1.0, scalar2=1.5, op0=ALU.mult, op1=ALU.add
    )
    r2 = pool.tile(shape, BF16, tag=f"{tag}_r2", name=f"{tag}_r2")
    nc.vector.tensor_mul(out=r2, in0=r, in1=t)
    return r2


def build_program():
    nc = bacc.Bacc(trn_type="TRN2", debug=False)

    x_d = nc.declare_dram_parameter("x", [S, IB, PB, PADH * PADW], BF16, isOutput=False)
    wt_d = nc.declare_dram_parameter("wT", [S, IB, PB, C_OUT, KK], BF16, isOutput=False)
    vecs_d = nc.declare_dram_parameter("vecs", [S, PB, NVEC], F32, isOutput=False)
    smat_d = nc.declare_dram_parameter("smat", [PB, 3, IB, C_OUT], F32, isOutput=False)
    g16_d = nc.declare_dram_parameter("g16", [OB, PB, 1], F32, isOutput=False)
    hotr_d = nc.declare_dram_parameter("hotr", [4, 3, PB], BF16, isOutput=False)
    y_d = nc.declare_dram_parameter("y", [S, OB, PB, HW], BF16, isOutput=True)

    with ExitStack() as ctx:
        tc = ctx.enter_context(tile.TileContext(nc))
        const = ctx.enter_context(tc.tile_pool(name="const", bufs=1))
        wpool = ctx.enter_context(tc.tile_pool(name="wmix", bufs=3))
        xrp = ctx.enter_context(tc.tile_pool(name="xpad", bufs=3))
        small = ctx.enter_context(tc.tile_pool(name="small", bufs=4))
        sq_p = ctx.enter_context(tc.tile_pool(name="sqp", bufs=4))
        ycp = ctx.enter_context(tc.tile_pool(name="ycpool", bufs=12))
        invp = ctx.enter_context(tc.tile_pool(name="invp", bufs=2))
        nsgp = ctx.enter_context(tc.tile_pool(name="nsgp", bufs=2))
        nstp = ctx.enter_context(tc.tile_pool(name="nstp", bufs=3))
        bcastp = ctx.enter_context(tc.tile_pool(name="bcast", bufs=4))
        outp = ctx.enter_context(tc.tile_pool(name="outs", bufs=3))
        dramp = ctx.enter_context(tc.tile_pool(name="dram", bufs=2, space="DRAM"))
        pconv = ctx.enter_context(tc.tile_pool(name="pconv", bufs=4, space="PSUM"))
        pnorm = ctx.enter_context(tc.tile_pool(name="pnorm", bufs=2, space="PSUM"))
        pbc = ctx.enter_context(tc.tile_pool(name="pbc", bufs=1, space="PSUM"))
        pdsq = ctx.enter_context(tc.tile_pool(name="pdsq", bufs=1, space="PSUM"))

        # ---- resident constants ----
        g16sb = [const.tile([PB, 1], F32, tag=f"g16_{ob}", name=f"g16_{ob}")
                 for ob in range(OB)]
        # one-hot column tiles for norm-row accumulation (lhsT of the
        # channel-sum matmul; row j of the PSUM [G, 512] gets the sum).
        # Pixel-tile groups per sample: s0 balanced, s1 tail-light with
        # shrinking groups (short exposed chain after the last matmul).
        GROUPS = {s: [4, 4] for s in range(S)}
        GROUPS[S - 1] = [5, 2, 1]
        # tail groups: fast bf16 Newton chains; B broadcasts via a bf16
        # DMA bounce (z stays on Pool), C via a PE ones-matmul (shortest
        # serial chain after the final conv matmul)
        FASTG = {(S - 1, 1): "fastdma", (S - 1, 2): "fastpe"}
        hots = {}
        for (s_, g_) in FASTG:
            G = GROUPS[s_][g_]
            for j in range(G):
                if (G, j) in hots:
                    continue
                hj = const.tile([PB, G], BF16, tag=f"hot{G}_{j}", name=f"hot{G}_{j}")
                nc.vector.memset(hj, 0.0)
                nc.vector.memset(hj[:, j:j + 1], 1.0)
                hots[(G, j)] = hj
        smat_t = const.tile([PB, 3, IB, C_OUT], F32, tag="smat", name="smat")
        # PE clock warm-up fodder: the p-state ramp (0.65/1.2GHz for the
        # first ~3us of activity) is paid by junk matmuls during the
        # DMA-bound head instead of by the first real conv chain
        junk = const.tile([PB, PT], BF16, tag="junk", name="junk")
        nc.vector.memset(junk, 0.0)
        # all-ones lhsT: the final tile's channel-sum matmul writes the
        # SAME sum into every output partition (cost is free-size only),
        # so the rsqrt chain's result is already broadcast -- no PE
        # ones-matmul hop, and z reads SBUF directly on the same engine
        ones128 = const.tile([PB, PB], BF16, tag="ones128", name="ones128")
        nc.vector.memset(ones128, 1.0)
        N_WARM = int(os.environ.get("KERNEL_WARM", "2"))
        # row-hot tiles: [G, 128] with row j all-ones -- lhsT of the
        # PE broadcast matmul (out[o,:] = inv[j,:]) for the tail groups.
        # Row-wise memsets can't start mid-partition; DMA'd from host.
        hotr = {}
        hidx = 0
        for (s_, g_), mode in sorted(FASTG.items()):
            if mode != "fastpe":
                continue
            G = GROUPS[s_][g_]
            for j in range(G):
                if (G, j) in hotr:
                    continue
                hr = const.tile([G, PB], BF16, tag=f"hotr{G}_{j}",
                                name=f"hotr{G}_{j}")
                nc.gpsimd.dma_start(out=hr, in_=hotr_d[hidx, 0:G])
                hotr[(G, j)] = hr
                hidx += 1

        HH = H // 2
        QC = HW // 4          # x quarter, in flat columns

        def prologue(s):
            """Per-sample setup: DMAs, weight mix, x pad-scale, demod.

            Emission order IS engine order (in-order engines): weights on
            the SP DMA stream, x + smat on the Pool stream; DVE does mix
            o-half-0, x quarter 0, demod rsqrt, then the rest.
            """
            st = {}
            vec = small.tile([PB, NVEC], F32, tag="vec", name="vec")
            nc.sync.dma_start(out=vec, in_=vecs_d[s])
            st["mpc"] = [vec[:, 2 + ib:3 + ib] for ib in range(IB)]
            m2k = [[vec[:, 4 + 3 * ib + k:5 + 3 * ib + k] for k in range(3)]
                   for ib in range(IB)]

            # host-premixed per-sample weights, straight into SBUF via
            # the SP stream: o-half 0 first (first conv chains), ib-major
            wmix = [wpool.tile([PB, C_OUT, KK], BF16, tag="wmix", name="wmix")
                    for _ in range(IB)]
            st["wmix"] = wmix
            for ib in range(IB):
                nc.sync.dma_start(
                    out=wmix[ib][:, 0:PB, :], in_=wt_d[s, ib, :, 0:PB, :],
                )
            xp = []
            for ib in range(IB):
                xp.append(xrp.tile([PB, PADH, PADW], BF16, tag="xpad",
                                   name="xpad"))
            st["xp"] = xp

            # x quarters: Pool stream for s0 (SP is busy with weights),
            # SP for later samples (Pool carries y writebacks by then).
            # s0's first quarter is split across BOTH streams so the two
            # ib blocks land (and the pad-copy can start) ~0.8us sooner.
            xdma = nc.gpsimd.dma_start if s == 0 else nc.sync.dma_start
            xfs = [xfp.tile([PB, HW], BF16, tag="xf", name="xf")
                   for _ in range(IB)]
            nc.gpsimd.dma_start(out=xfs[0][:, 0:QC], in_=x_d[s, 0, :, 0:QC])
            for ib in range(IB):
                xr = xp[ib]
                nc.gpsimd.memset(xr[:, 0:1, :], 0.0)
                nc.gpsimd.memset(xr[:, PADH - 1:PADH, :], 0.0)
                nc.gpsimd.memset(xr[:, 1:H + 1, 0:1], 0.0)
                nc.gpsimd.memset(xr[:, 1:H + 1, PADW - 1:PADW], 0.0)
            (nc.sync.dma_start if s == 0 else nc.gpsimd.dma_start)(
                out=xfs[1][:, 0:QC], in_=x_d[s, 1, :, 0:QC])
            if s == 0:
                # demod stats + gamma, one transfer each, on Pool
                nc.gpsimd.dma_start(out=smat_t, in_=smat_d[0:PB])
                nc.gpsimd.dma_start(out=g16sb[0], in_=g16_d[0])
                nc.gpsimd.dma_start(out=g16sb[1], in_=g16_d[1])
            for ib in range(IB):
                nc.sync.dma_start(
                    out=wmix[ib][:, PB:C_OUT, :], in_=wt_d[s, ib, :, PB:C_OUT, :],
                )
            for q in range(1, 4):
                for ib in range(IB):
                    xdma(out=xfs[ib][:, q * QC:(q + 1) * QC],
                         in_=x_d[s, ib, :, q * QC:(q + 1) * QC])

            st["m2k"] = m2k

            if s > 0:
                emit_demod(st)

            for q in range(1, 4):
                r0, r1 = XQ[q]
                for ib in range(IB):
                    xdma(out=xp[ib][:, r0:r1, :],
                         in_=x_d[s, ib, :, r0 * PADW:r1 * PADW])
            return st

        def emit_demod(st):
            """Demod matvecs (PE) + d/gd columns (DVE).

            Deferred past the first conv chain for sample 0 so the
            PE's in-order queue isn't blocked waiting on smat."""
            m2k = st["m2k"]
            dsq_ps = [pdsq.tile([PB, 1], F32, tag="dsq", name="dsq")
                      for _ in range(OB)]
            for ob in range(OB):
                i_mv = 0
                for ib in range(IB):
                    for k in range(3):
                        nc.tensor.matmul(
                            dsq_ps[ob],
                            lhsT=smat_t[:, k, ib, ob * PB:(ob + 1) * PB],
                            rhs=m2k[ib][k],
                            start=(i_mv == 0), stop=(i_mv == 3 * IB - 1),
                        )
                        i_mv += 1
            dcol, gdcol = [], []
            for ob in range(OB):
                d = _rsqrt_dve(nc, small, dsq_ps[ob], EPS, [PB, 1], f"d{ob}",
                               iters=2)
                dcol.append(d)
                gd = small.tile([PB, 1], F32, tag=f"gd{ob}", name=f"gd{ob}")
                nc.vector.tensor_mul(out=gd, in0=d, in1=g16sb[ob])
                gdcol.append(gd)
            st["dcol"], st["gdcol"] = dcol, gdcol

        def conv_group(s, st, g):
            """Emit conv + square + channel-norm sums for pixel group g.

            Tail groups accumulate the 256-channel sums on PE (one-hot
            matmuls into PSUM [G, 512]); steady groups use Pool's
            partition_all_reduce + add + a row-gather DMA into an SBUF
            [G, 512], keeping the PE free for conv.
            """
            G = GROUPS[s][g]
            g0 = sum(GROUPS[s][:g])
            mode = FASTG.get((s, g), "slow")
            if mode == "slow":
                nsum = nsgp.tile([G, PT], BF16, tag="nsg", name="nsg")
            elif mode == "fastpe":
                nsum = pnorm.tile([PB, PT], F32, tag="nsum", name="nsum")
            else:
                nsum = pnorm.tile([G, PT], F32, tag="nsum", name="nsum")
            ycs = {}
            for lpt in range(G):
                pt = g0 + lpt
                pss = []
                for ob in range(OB):
                    if s == 0 and g == 0 and lpt == 0 and ob == 0 and N_WARM:
                        wps = pconv.tile([PB, PT], F32, tag="conv", name="conv")
                        for i in range(N_WARM):
                            nc.tensor.matmul(
                                wps[0:2, :], lhsT=hots[(2, 0)], rhs=junk,
                                start=(i == 0), stop=(i == N_WARM - 1),
                            )
                    ps = pconv.tile([PB, PT], F32, tag="conv", name="conv")
                    pss.append(ps)
                    n_mm = IB * KK
                    i_mm = 0
                    for ib in range(IB):
                        for ki in range(K):
                            for kj in range(K):
                                lhsT = st["wmix"][ib][
                                    :, ob * PB:(ob + 1) * PB, ki * K + kj]
                                rhs = st["xp"][ib][
                                    :,
                                    pt * ROWS_PT + ki: pt * ROWS_PT + ki + ROWS_PT,
                                    kj: kj + W,
                                ]
                                nc.tensor.matmul(
                                    ps, lhsT=lhsT, rhs=rhs,
                                    start=(i_mm == 0), stop=(i_mm == n_mm - 1),
                                )
                                i_mm += 1
                    if s == 0 and g == 0 and lpt == 0 and ob == 0:
                        # demod slots in behind the first conv chain
                        emit_demod(st)
                # ACT order: both squares first (they gate the norm-sum
                # matmuls and, at the tail, the final rsqrt chain)
                sqs = []
                for ob in range(OB):
                    sq = sq_p.tile([PB, PT], BF16, tag="sq", name="sq")
                    nc.scalar.activation(
                        out=sq, in_=pss[ob], func=AF.Square, scale=st["dcol"][ob]
                    )
                    sqs.append(sq)
                if mode == "slow":
                    for ob in range(OB):
                        nc.gpsimd.partition_all_reduce(
                            sqs[ob][:], sqs[ob][:], PB, bass_isa.ReduceOp.add
                        )
                    nst = nstp.tile([1, PT], BF16, tag="nst", name="nst")
                    nc.gpsimd.tensor_add(
                        out=nst, in0=sqs[0][0:1, :], in1=sqs[1][0:1, :]
                    )
                    nc.sync.dma_start(out=nsum[lpt:lpt + 1, :], in_=nst)
                elif mode == "fastpe":
                    for ob in range(OB):
                        nc.tensor.matmul(
                            nsum, lhsT=ones128, rhs=sqs[ob],
                            start=(ob == 0), stop=(ob == OB - 1),
                        )
                else:
                    for ob in range(OB):
                        nc.tensor.matmul(
                            nsum, lhsT=hots[(G, lpt)], rhs=sqs[ob],
                            start=(lpt == 0 and ob == 0),
                            stop=(lpt == G - 1 and ob == OB - 1),
                        )
                for ob in range(OB):
                    # y*d*gamma*sqrt(C) kept fp32 for the output path
                    yc = ycp.tile([PB, PT], F32, tag="yc", name="yc")
                    nc.scalar.activation(
                        out=yc, in_=pss[ob], func=AF.Copy, scale=st["gdcol"][ob]
                    )
                    ycs[(lpt, ob)] = yc
            return nsum, ycs

        def finish_group(s, g, nsum, ycs, st=None):
            """rsqrt of the norm rows + z/SiLU/writeback for group g.

            Steady state broadcasts inv rows p0->all via a DRAM bounce
            (latency hidden under conv); the tail groups (FASTG) use a
            PE ones-matmul broadcast instead (DMA sem latency is ~1.6us
            per hop, matmul is ~0.3us total).
            """
            G = GROUPS[s][g]
            g0 = sum(GROUPS[s][:g])
            mode = FASTG.get((s, g), "slow")
            if mode == "fastpe":
                p = g0
                inv = _rsqrt_bf16(nc, invp, nsum, RT_CLAMP, [PB, PT], "nrmf")
                for ob in range(OB):
                    # inv is already all-partition SBUF: same-engine z,
                    # zero cross-engine hops after the chain
                    z = outp.tile([PB, PT], F32, tag="z", name="z")
                    nc.vector.tensor_mul(out=z, in0=ycs[(0, ob)], in1=inv)
                    yo = outp.tile([PB, PT], BF16, tag="yo", name="yo")
                    if SIM_SILU:
                        nc.scalar.activation(out=yo, in_=z, func=AF.Sigmoid)
                        nc.vector.tensor_mul(out=yo, in0=z, in1=yo)
                    else:
                        nc.scalar.activation(out=yo, in_=z, func=AF.Silu)
                    nc.sync.dma_start(
                        out=y_d[s, ob, :, p * PT:(p + 1) * PT], in_=yo,
                    )
                return
            if mode == "slow":
                inv = _rsqrt_dve(nc, invp, nsum, RT_CLAMP, [G, PT], "nrm",
                                 iters=1)
            else:
                inv = _rsqrt_bf16(nc, invp, nsum, RT_CLAMP, [G, PT], "nrmf")
            idt = BF16 if mode == "fastdma" else F32
            dinv = dramp.tile([G, PT], idt, tag="dinv", name="dinv")
            nc.sync.dma_start(out=dinv, in_=inv)
            last = (s == S - 1 and g == len(GROUPS[s]) - 1)
            for lpt in range(G):
                p = g0 + lpt
                invb = bcastp.tile([PB, PT], idt, tag="invb", name="invb")
                nc.sync.dma_start(
                    out=invb, in_=dinv[lpt:lpt + 1, :].to_broadcast((PB, PT)),
                )
                for ob in range(OB):
                    z = outp.tile([PB, PT], F32, tag="z", name="z")
                    if False:
                        pass
                    else:
                        # all-SBUF operands -> Pool (427ns, cheaper than
                        # DVE and off the critical engines)
                        nc.gpsimd.tensor_mul(out=z, in0=ycs[(lpt, ob)], in1=invb)
                    yo = outp.tile([PB, PT], BF16, tag="yo", name="yo")
                    if SIM_SILU:
                        nc.scalar.activation(out=yo, in_=z, func=AF.Sigmoid)
                        nc.vector.tensor_mul(out=yo, in0=z, in1=yo)
                    else:
                        nc.scalar.activation(out=yo, in_=z, func=AF.Silu)
                    ydma = nc.sync.dma_start if last else nc.gpsimd.dma_start
                    ydma(out=y_d[s, ob, :, p * PT:(p + 1) * PT], in_=yo)

        # ---- main schedule: the prologue of sample s+1 is emitted before
        # the deferred finish of sample s's final group, so the in-order
        # DVE/PE streams roll into the next sample without idling. On the
        # last sample the final two (small) groups' finishes are emitted
        # after ALL conv work, so PE never waits on a norm chain.
        st = prologue(0)
        for s in range(S):
            ngroups = len(GROUPS[s])
            if s < S - 1:
                for g in range(ngroups):
                    nsum, ycs = conv_group(s, st, g)
                    if g == ngroups - 1:
                        nst = prologue(s + 1)
                        finish_group(s, g, nsum, ycs)
                        st = nst
                    else:
                        finish_group(s, g, nsum, ycs)
            else:
                pend = []
                for g in range(ngroups):
                    nsum, ycs = conv_group(s, st, g)
                    if g < ngroups - 2:
                        finish_group(s, g, nsum, ycs)
                    else:
                        pend.append((g, nsum, ycs))
                for g, nsum, ycs in pend:
                    finish_group(s, g, nsum, ycs, st=st)
    nc.finalize()
    return nc


_NC_CACHE = {}


def _get_program():
    if "nc" not in _NC_CACHE:
        _NC_CACHE["nc"] = build_program()
    return _NC_CACHE["nc"]


def _host_prep(x, mod, kernel_mod, weights, gamma):
    import ml_dtypes

    x = np.asarray(x, dtype=np.float32)
    mod = np.asarray(mod, dtype=np.float32)
    kernel_mod = np.asarray(kernel_mod, dtype=np.float32)
    weights = np.asarray(weights, dtype=np.float32)
    gamma = np.asarray(gamma, dtype=np.float32)

    # softmax over the (tiny) kernel bank dim
    e = np.exp(kernel_mod - kernel_mod.max(axis=-1, keepdims=True))
    attn = (e / e.sum(axis=-1, keepdims=True)).astype(np.float32)     # [B, NK]

    modp1 = mod + 1.0                                                 # [B, C_IN]
    m2 = modp1 * modp1

    # [NK, O, I, K, K] -> [NK, I, O, K*K]; per-sample softmax mix done on
    # the host (fp32) so the device never touches the raw bank
    wTf = weights.transpose(0, 2, 1, 3, 4).reshape(NK, IB, PB, C_OUT, KK)
    # bank Gram stats over kk: S00, S01, S11 as [i, o], split by i-block
    wio = weights.transpose(0, 2, 1, 3, 4).reshape(NK, C_IN, C_OUT, KK)
    s00 = (wio[0] * wio[0]).sum(-1)
    s01 = (wio[0] * wio[1]).sum(-1)
    s11 = (wio[1] * wio[1]).sum(-1)
    smat = np.ascontiguousarray(
        np.stack([s00, s01, s11]).reshape(3, IB, PB, C_OUT)
        .transpose(2, 0, 1, 3).astype(np.float32)
    )
    g16 = np.ascontiguousarray(
        (gamma * np.sqrt(C_OUT)).astype(np.float32).reshape(OB, PB, 1)
    )
    # row-hot lhsT patterns for the tail PE-broadcast: slot i = one-hot
    # row i (slot 0 also serves the G=1 all-ones case); consumed in
    # build_program's sorted-FASTG "fastpe" order
    hotr = np.zeros((4, 3, PB), ml_dtypes.bfloat16)
    hotr[0, 0, :] = 1
    hotr[1, 1, :] = 1
    hotr[2, 2, :] = 1
    hotr[3, 0, :] = 1

    in_maps = []
    for c in range(N_CORES):
        sl = slice(c * S, (c + 1) * S)
        vecs = np.empty((S, PB, NVEC), np.float32)
        for si in range(S):
            b = c * S + si
            a0, a1 = attn[b, 0], attn[b, 1]
            vecs[si, :, 0] = a0
            vecs[si, :, 1] = a1
            vecs[si, :, 2:2 + IB] = modp1[b].reshape(IB, PB).T
            m2b = m2[b].reshape(IB, PB)
            for ib in range(IB):
                vecs[si, :, 4 + 3 * ib] = m2b[ib] * (a0 * a0)
                vecs[si, :, 5 + 3 * ib] = m2b[ib] * (2.0 * a0 * a1)
                vecs[si, :, 6 + 3 * ib] = m2b[ib] * (a1 * a1)
        wmix = np.ascontiguousarray(
            attn[sl, 0, None, None, None, None] * wTf[0][None]
            + attn[sl, 1, None, None, None, None] * wTf[1][None]
        ).astype(ml_dtypes.bfloat16)                    # [S, IB, PB, C_OUT, KK]
        xpad = np.zeros((S, IB, PB, PADH, PADW), np.float32)
        xpad[:, :, :, 1:H + 1, 1:W + 1] = (
            x[sl] * modp1[sl, :, None, None]
        ).reshape(S, IB, PB, H, W)
        in_maps.append({
            "x": xpad.reshape(S, IB, PB, PADH * PADW).astype(ml_dtypes.bfloat16),
            "wT": wmix,
            "smat": smat,
            "hotr": hotr,
            "vecs": vecs,
            "g16": g16,
        })
    return in_maps


def kernel(x, mod, kernel_mod, weights, gamma, _trace=False, _trace_kwargs=None):
    nc = _get_program()
    in_maps = _host_prep(x, mod, kernel_mod, weights, gamma)
    res = run_bass_kernel_spmd(
        nc, in_maps, list(range(N_CORES)),
        trace=_trace, **(_trace_kwargs or {}),
    )
    y = np.concatenate(
        [np.asarray(res.results[c]["y"]).astype(np.float32).reshape(S, C_OUT, H, W)
         for c in range(N_CORES)],
        axis=0,
    )
    if _trace:
        kernel.last_results = res
    return y


kernel.last_results = None
